# revision 1
# baseline (speedup 1.0000x reference)
"""AdaFocal loss (BCE + focal reweighting via 15-bin gamma table) on 8 TRN2 cores.

Math (per element, u = (2t-1)*x):
    pt  = sigmoid(u)
    ce  = softplus(-u) = -log(pt)
    bin = clip(floor(pt*15), 0, 14); g = bin_gammas[bin]
    loss = ce * (1 - sign(g)*pt + EPS) ** |g|
Output = sum(loss).

Device formulation uses only the natural_log_exp activation-table set:
    v  = exp(-u)          (exp, scale=-2 on u2 = (t-0.5)*x)
    ce = ln(1 + v)        (ln with bias=1)
    w  = exp(-ce) = pt    (exact identity: e^{-ln(1+v)} = 1/(1+v) = sigmoid(u))
Fast path (all gammas == 1, the shipped configuration):
    loss = ce*(1 - w + EPS)  ->  accumulate (w - (1+EPS))*ce = -loss on DVE.
General path handles an arbitrary gamma table via per-bin masks.

Sharding: pure data parallel over the batch dim; each of the 8 cores gets
2048 rows. Each core returns per-partition partial sums; the host sums them.
"""

import sys

if "/opt/trn_rl_repo" not in sys.path:
    sys.path.insert(0, "/opt/trn_rl_repo")

import numpy as np

R, C = 16384, 2048
NCORES = 8
SHARD_ELEMS = (R // NCORES) * C  # 4,194,304 per core
P = 128
F = 4096
NT = SHARD_ELEMS // (P * F)  # 8 tiles per core
EPS = float(np.finfo(np.float32).eps)
NUM_BINS = 15

_cache = {}


def _build_fast():
    from concourse import bacc, tile, mybir

    nc = bacc.Bacc("TRN2", target_bir_lowering=False, debug=False, num_devices=NCORES)
    x_d = nc.dram_tensor("x", [NT, P, F], mybir.dt.float32, kind="ExternalInput")
    t_d = nc.dram_tensor("t", [NT, P, F], mybir.dt.int32, kind="ExternalInput")
    out_d = nc.dram_tensor("out", [P, NT], mybir.dt.float32, kind="ExternalOutput")

    with tile.TileContext(nc) as tc:
        with (
            tc.tile_pool(name="accp", bufs=1) as accp,
            tc.tile_pool(name="sbuf", bufs=2) as pool,
        ):
            acc = accp.tile([P, NT], mybir.dt.float32)
            for r in range(NT):
                xt = pool.tile([P, F], mybir.dt.float32, tag="x")
                tt = pool.tile([P, F], mybir.dt.int32, tag="t")
                nc.sync.dma_start(out=xt[:, :], in_=x_d[r, :, :])
                nc.sync.dma_start(out=tt[:, :], in_=t_d[r, :, :])
                u2 = pool.tile([P, F], mybir.dt.float32, tag="u2")
                nc.vector.scalar_tensor_tensor(
                    out=u2[:, :], in0=tt[:, :], scalar=0.5, in1=xt[:, :],
                    op0=mybir.AluOpType.subtract, op1=mybir.AluOpType.mult)
                v = pool.tile([P, F], mybir.dt.float32, tag="v")
                nc.scalar.activation(
                    v[:, :], u2[:, :], mybir.ActivationFunctionType.Exp, scale=-2.0)
                ce = pool.tile([P, F], mybir.dt.float32, tag="ce")
                nc.scalar.activation(
                    ce[:, :], v[:, :], mybir.ActivationFunctionType.Ln, bias=1.0)
                w = pool.tile([P, F], mybir.dt.float32, tag="w")
                nc.scalar.activation(
                    w[:, :], ce[:, :], mybir.ActivationFunctionType.Exp, scale=-1.0)
                junk = pool.tile([P, F], mybir.dt.float32, tag="v")
                nc.vector.scalar_tensor_tensor(
                    out=junk[:, :], in0=w[:, :], scalar=1.0 + EPS, in1=ce[:, :],
                    op0=mybir.AluOpType.subtract, op1=mybir.AluOpType.mult,
                    accum_out=acc[:, r:r + 1])
            nc.sync.dma_start(out=out_d[:, :], in_=acc[:, :])

    nc.compile()
    return nc


def _build_general():
    """Arbitrary gamma table: per-element gamma via 15 masked accumulations.

    g table arrives pre-broadcast to [P, 15] (host tiles it), along with
    per-partition sign/abs columns.
    """
    from concourse import bacc, tile, mybir

    nc = bacc.Bacc("TRN2", target_bir_lowering=False, debug=False, num_devices=NCORES)
    x_d = nc.dram_tensor("x", [NT, P, F], mybir.dt.float32, kind="ExternalInput")
    t_d = nc.dram_tensor("t", [NT, P, F], mybir.dt.int32, kind="ExternalInput")
    g_d = nc.dram_tensor("g", [P, NUM_BINS], mybir.dt.float32, kind="ExternalInput")
    out_d = nc.dram_tensor("out", [P, NT], mybir.dt.float32, kind="ExternalOutput")

    with tile.TileContext(nc) as tc:
        with (
            tc.tile_pool(name="constp", bufs=1) as constp,
            tc.tile_pool(name="sbuf", bufs=2) as pool,
        ):
            acc = constp.tile([P, NT], mybir.dt.float32)
            g_sb = constp.tile([P, NUM_BINS], mybir.dt.float32)
            gs_sb = constp.tile([P, NUM_BINS], mybir.dt.float32)
            gm_sb = constp.tile([P, NUM_BINS], mybir.dt.float32)
            nc.sync.dma_start(out=g_sb[:, :], in_=g_d[:, :])
            nc.scalar.activation(
                gs_sb[:, :], g_sb[:, :], mybir.ActivationFunctionType.Sign)
            nc.scalar.activation(
                gm_sb[:, :], g_sb[:, :], mybir.ActivationFunctionType.Abs)
            for r in range(NT):
                xt = pool.tile([P, F], mybir.dt.float32, tag="x")
                tt = pool.tile([P, F], mybir.dt.int32, tag="t")
                nc.sync.dma_start(out=xt[:, :], in_=x_d[r, :, :])
                nc.sync.dma_start(out=tt[:, :], in_=t_d[r, :, :])
                u2 = pool.tile([P, F], mybir.dt.float32, tag="u2")
                nc.vector.scalar_tensor_tensor(
                    out=u2[:, :], in0=tt[:, :], scalar=0.5, in1=xt[:, :],
                    op0=mybir.AluOpType.subtract, op1=mybir.AluOpType.mult)
                v = pool.tile([P, F], mybir.dt.float32, tag="v")
                nc.scalar.activation(
                    v[:, :], u2[:, :], mybir.ActivationFunctionType.Exp, scale=-2.0)
                ce = pool.tile([P, F], mybir.dt.float32, tag="ce")
                nc.scalar.activation(
                    ce[:, :], v[:, :], mybir.ActivationFunctionType.Ln, bias=1.0)
                w = pool.tile([P, F], mybir.dt.float32, tag="w")
                nc.scalar.activation(
                    w[:, :], ce[:, :], mybir.ActivationFunctionType.Exp, scale=-1.0)
                # bin index: b = round_to_int(w*15 - 0.5) == floor(w*15) a.e.
                bf = pool.tile([P, F], mybir.dt.float32, tag="bf")
                nc.vector.tensor_scalar(
                    out=bf[:, :], in0=w[:, :], scalar1=float(NUM_BINS),
                    scalar2=0.5, op0=mybir.AluOpType.mult,
                    op1=mybir.AluOpType.subtract)
                bi = pool.tile([P, F], mybir.dt.int32, tag="bi")
                nc.vector.tensor_scalar(
                    out=bi[:, :], in0=bf[:, :], scalar1=0.0,
                    scalar2=float(NUM_BINS - 1), op0=mybir.AluOpType.max,
                    op1=mybir.AluOpType.min)
                # gamma gather via 15 masked accumulations
                gam = pool.tile([P, F], mybir.dt.float32, tag="gam")
                gsel = pool.tile([P, F], mybir.dt.float32, tag="gsel")
                tmp = pool.tile([P, F], mybir.dt.float32, tag="tmp")
                nc.vector.tensor_scalar(
                    out=gam[:, :], in0=bi[:, :], scalar1=0,
                    scalar2=gm_sb[:, 0:1], op0=mybir.AluOpType.is_equal,
                    op1=mybir.AluOpType.mult)
                nc.vector.tensor_scalar(
                    out=gsel[:, :], in0=bi[:, :], scalar1=0,
                    scalar2=gs_sb[:, 0:1], op0=mybir.AluOpType.is_equal,
                    op1=mybir.AluOpType.mult)
                for k in range(1, NUM_BINS):
                    nc.vector.tensor_scalar(
                        out=tmp[:, :], in0=bi[:, :], scalar1=k,
                        scalar2=gm_sb[:, k:k + 1], op0=mybir.AluOpType.is_equal,
                        op1=mybir.AluOpType.mult)
                    nc.vector.tensor_tensor(
                        out=gam[:, :], in0=gam[:, :], in1=tmp[:, :],
                        op=mybir.AluOpType.add)
                    nc.vector.tensor_scalar(
                        out=tmp[:, :], in0=bi[:, :], scalar1=k,
                        scalar2=gs_sb[:, k:k + 1], op0=mybir.AluOpType.is_equal,
                        op1=mybir.AluOpType.mult)
                    nc.vector.tensor_tensor(
                        out=gsel[:, :], in0=gsel[:, :], in1=tmp[:, :],
                        op=mybir.AluOpType.add)
                # base = 1 + EPS - gs*w ; L = ln(base); e = exp(gm*L)
                base = pool.tile([P, F], mybir.dt.float32, tag="base")
                nc.vector.tensor_tensor(
                    out=base[:, :], in0=gsel[:, :], in1=w[:, :],
                    op=mybir.AluOpType.mult)
                nc.vector.tensor_scalar(
                    out=base[:, :], in0=base[:, :], scalar1=-1.0,
                    scalar2=1.0 + EPS, op0=mybir.AluOpType.mult,
                    op1=mybir.AluOpType.add)
                lnb = pool.tile([P, F], mybir.dt.float32, tag="lnb")
                nc.scalar.activation(
                    lnb[:, :], base[:, :], mybir.ActivationFunctionType.Ln)
                m = pool.tile([P, F], mybir.dt.float32, tag="m")
                nc.vector.tensor_tensor(
                    out=m[:, :], in0=gam[:, :], in1=lnb[:, :],
                    op=mybir.AluOpType.mult)
                powr = pool.tile([P, F], mybir.dt.float32, tag="powr")
                nc.scalar.activation(
                    powr[:, :], m[:, :], mybir.ActivationFunctionType.Exp)
                junk = pool.tile([P, F], mybir.dt.float32, tag="m")
                nc.vector.scalar_tensor_tensor(
                    out=junk[:, :], in0=powr[:, :], scalar=0.0, in1=ce[:, :],
                    op0=mybir.AluOpType.add, op1=mybir.AluOpType.mult,
                    accum_out=acc[:, r:r + 1])
            nc.sync.dma_start(out=out_d[:, :], in_=acc[:, :])

    nc.compile()
    return nc


def _get(which):
    if which not in _cache:
        _cache[which] = _build_fast() if which == "fast" else _build_general()
    return _cache[which]


def _run(inputs, targets, bin_gammas, trace=False, **spmd_kwargs):
    from concourse.bass_utils import run_bass_kernel_spmd

    xs = np.ascontiguousarray(inputs).reshape(NCORES, NT, P, F)
    ts = np.ascontiguousarray(targets).reshape(NCORES, NT, P, F)
    fast = bool(np.all(bin_gammas == 1.0))
    nc = _get("fast" if fast else "general")
    if fast:
        in_maps = [{"x": xs[i], "t": ts[i]} for i in range(NCORES)]
    else:
        g_full = np.tile(
            np.asarray(bin_gammas, dtype=np.float32).reshape(1, NUM_BINS), (P, 1))
        in_maps = [{"x": xs[i], "t": ts[i], "g": g_full} for i in range(NCORES)]
    res = run_bass_kernel_spmd(
        nc, in_maps, core_ids=list(range(NCORES)), trace=trace, **spmd_kwargs)
    partials = np.stack([r["out"] for r in res.results])
    if fast:
        total = -partials.astype(np.float64).sum()
    else:
        total = partials.astype(np.float64).sum()
    return np.float32(total), res


def kernel(inputs, targets, bin_gammas):
    total, _ = _run(inputs, targets, bin_gammas)
    return total
